# revision 25
# baseline (speedup 1.0000x reference)
"""Trainium2 Bass kernel for FFNDynamicMemories (SwiGLU FFN + per-token
hypernetwork-generated low-rank memory path).

Sharding strategy (8 cores):
  - The dominant cost is gen_w2 [3*R*DIM=98304, GEN_HIDDEN=512] (192 MiB fp32):
    it is sharded across cores by *generated-weight rows*: core c owns
    r in [8c, 8c+8) of W1_m and W2_m rows, plus the matching r-slice of W3_m
    (reordered r-major on the host so each 512-row block is one r).
    Each core therefore reads only 1/8 of gen_w2.
  - hg = silu(gen_w1 @ m_tok) is computed by every core for all tokens (tiny).
  - The base SwiGLU FFN is sharded by hidden dim (256 of 2048 per core).
  - Each core produces a full-shape partial y; one on-device ReduceScatter
    (over the token axis) combines them; the host just concatenates the
    8 [128, 512] shards.

Compute mapping per core:
  - Generation matmuls run in fp8e4 (e4m3) with MatmulPerfMode.DoubleRow
    (2 K-rows per free-dim slot, ~1.4x the bf16/fp32r PE rate at FD=512,
    and 4x less HBM traffic for the gen_w2 stream).  gen_w2 is host-scaled
    by 4096 so its ~1e-3-rms values land mid-range in e4m3; the u-path
    compensates with x/4096 (host) and the y_mem path with
    sigmoid(gate)/4096 folded into the per-token h_mem scalars.  The
    mem-path output is ~0.1% of the final scale, so fp8's ~4% error there
    contributes ~3e-4 relative — same order as the fp32r baseline.
    w_tile[t_block 128, 512 rows] = hgT_pair.T @ gen_w2T_pair, accumulated
    over 2 h-pair-chunks (each K=256) in PSUM.
  - The per-token "apply" (u1/u2 row-dot with x, y_mem accumulation over r)
    uses fused DVE ops reading the generated tile straight from PSUM:
      u[t, r]   = tensor_tensor_reduce(w_psum * x, +, init=x@gen_b2_slice)
      y3acc[t]  += scalar_tensor_tensor(w_psum * h_mem[t, r]) (in-place add)
  - gen_w2 row-blocks stream in the OUTER loop (tokens inner) so the big
    tensor is read from HBM exactly once.
"""

import sys

sys.path.insert(0, "/opt/trn_rl_repo")

import numpy as np

# Problem constants (hardcoded per the harness contract).
B, T, DIM, HIDDEN, D_M, R = 2, 512, 512, 2048, 256, 64
GEN_HIDDEN = 2 * D_M  # 512
NTOK = B * T  # 1024
RD = R * DIM  # 32768
NCORES = 8
R_LOC = R // NCORES  # 8 r values per core
HID_LOC = HIDDEN // NCORES  # 256 hidden per core
P = 128
NTB = NTOK // P  # 8 token blocks
N_RB = 3 * R_LOC  # 24 row-blocks of 512 generated rows per core
KC_H = GEN_HIDDEN // P  # 4 h-chunks
KC_D = DIM // P  # 4 d-chunks
KC_DM = D_M // P  # 2 d_m chunks
KC_HID = HID_LOC // P  # 2 hidden chunks

_CACHE = {}


def _build_program(gen_bias_zero=False, b3_zero=False):
    """gen_bias_zero/b3_zero: value-specialization flags (checked by kernel()
    against the actual inputs; a mismatch rebuilds the program). When gen_b2
    is all-zero the u/y_mem bias-correction matmuls, the h_mem transpose and
    the y3corr matmul are elided; when b3 is all-zero its K=1 row matmul is."""
    import concourse.bass as bass
    import concourse.mybir as mybir
    import concourse.tile as tile
    from concourse import bacc
    from concourse.masks import make_identity

    f32 = mybir.dt.float32
    f32r = mybir.dt.float32r
    f8 = mybir.dt.float8e4
    DR = mybir.MatmulPerfMode.DoubleRow
    Act = mybir.ActivationFunctionType
    Alu = mybir.AluOpType

    nc = bacc.Bacc("TRN2", target_bir_lowering=False, debug=False,
                   num_devices=NCORES)

    # ---- DRAM parameters (per-core shapes) ----
    x_d = nc.dram_tensor("x", [NTOK, DIM], f32, kind="ExternalInput")
    xt_d = nc.dram_tensor("xt", [DIM, NTOK], f32r, kind="ExternalInput")
    mtt_d = nc.dram_tensor("mtt", [D_M, NTOK], f32r, kind="ExternalInput")
    gw1t_d = nc.dram_tensor("gw1t", [D_M, GEN_HIDDEN], f32r, kind="ExternalInput")
    gb1_d = nc.dram_tensor("gb1", [GEN_HIDDEN], f32, kind="ExternalInput")
    # packed gen_w2 shard: [rb, p(h within chunk), hc, 512 rows], fp8 x4096
    g2p_d = nc.dram_tensor("g2p", [N_RB, P, KC_H, 512], f8, kind="ExternalInput")
    b1c_d = nc.dram_tensor("b1c", [KC_D, P, R_LOC], f32r, kind="ExternalInput")
    b2c_d = nc.dram_tensor("b2c", [KC_D, P, R_LOC], f32r, kind="ExternalInput")
    b3ct_d = nc.dram_tensor("b3ct", [R_LOC, DIM], f32r, kind="ExternalInput")
    w1t_d = nc.dram_tensor("w1t", [DIM, HID_LOC], f32r, kind="ExternalInput")
    w2t_d = nc.dram_tensor("w2t", [DIM, HID_LOC], f32r, kind="ExternalInput")
    w3t_d = nc.dram_tensor("w3t", [HID_LOC, DIM], f32r, kind="ExternalInput")
    bb1_d = nc.dram_tensor("bb1", [HID_LOC], f32, kind="ExternalInput")
    bb2_d = nc.dram_tensor("bb2", [HID_LOC], f32, kind="ExternalInput")
    b3_d = nc.dram_tensor("b3", [DIM], f32, kind="ExternalInput")
    gate_d = nc.dram_tensor("gate", [1, 1], f32, kind="ExternalInput")
    import os as _os
    _skip_rs = bool(int(_os.environ.get("KERNEL_SKIP_RS", "0")))
    _exact_silu = bool(int(_os.environ.get("KERNEL_EXACT_SILU", "0")))
    # ReduceScatter chunking: 8 = one collective per finished token block
    # (overlaps all but the last chunk with pass-B compute); 1 = single RS
    _rs_chunks = int(_os.environ.get("KERNEL_RS_CHUNKS", "8"))
    RS_O = P // NCORES  # 16 output rows per core per RS chunk
    # A/B probe: disable DoubleRow (plain fp8 matmuls, 4 K-chunks) to see
    # whether the real PE rate binds the measured time
    _no_dr = bool(int(_os.environ.get("KERNEL_NO_DR", "0")))
    if _skip_rs:
        y_out_d = nc.dram_tensor("y_out", [NTOK, DIM], f32, kind="ExternalOutput")
    else:
        y_out_d = nc.dram_tensor("y_out", [P, DIM], f32, kind="ExternalOutput")

    with tile.TileContext(nc) as tc:
        with (
            tc.tile_pool(name="const", bufs=1) as cpool,
            tc.tile_pool(name="g2s", bufs=4) as g2pool,
            tc.tile_pool(name="scr", bufs=3) as spool,
            tc.tile_pool(name="outp", bufs=3) as opool,
            tc.tile_pool(name="dram", bufs=1, space="DRAM") as dpool,
        ):
            # ---- resident SBUF tiles ----
            x_sb = cpool.tile([P, NTB, DIM], f32)
            xt_sb = cpool.tile([P, KC_D, NTOK], f32r)
            mtt_sb = cpool.tile([P, KC_DM, NTOK], f32r)
            gw1t_sb = cpool.tile([P, KC_DM, GEN_HIDDEN], f32r)
            gb1_sb = cpool.tile([P, KC_H], f32)
            w1t_sb = cpool.tile([P, KC_D, HID_LOC], f32r)
            w2t_sb = cpool.tile([P, KC_D, HID_LOC], f32r)
            w3t_sb = cpool.tile([P, KC_HID, DIM], f32r)
            bb1_sb = cpool.tile([P, KC_HID], f32)
            bb2_sb = cpool.tile([P, KC_HID], f32)
            b3_sb = cpool.tile([1, DIM], f32)
            b3r_sb = cpool.tile([1, DIM], f32r)
            b1c_sb = cpool.tile([P, KC_D, R_LOC], f32r)
            b2c_sb = cpool.tile([P, KC_D, R_LOC], f32r)
            b3ct_sb = cpool.tile([R_LOC, DIM], f32r)
            gate_sb = cpool.tile([P, 1], f32)
            ident_sb = cpool.tile([P, P], f32)
            ones1f_sb = cpool.tile([1, P], f32)
            ones1_sb = cpool.tile([1, P], f32r)
            hgt_sb = cpool.tile([P, KC_H, NTOK], f8)
            ht_sb = cpool.tile([P, KC_HID, NTOK], f32r)
            ub1_sb = cpool.tile([P, NTB, R_LOC], f32)
            ub2_sb = cpool.tile([P, NTB, R_LOC], f32)
            u1_sb = cpool.tile([P, NTB, R_LOC], f32)
            u2_sb = cpool.tile([P, NTB, R_LOC], f32)
            hmg_sb = cpool.tile([P, NTB, R_LOC], f32)
            hmt_sb = cpool.tile([R_LOC, NTB, P], f32r)
            # y3 accumulates in bf16: the ACT engine drains each pass-B psum
            # tile (scaled by hmg) to bf16, and DVE adds run at the 2x
            # 16-bit rate; y_mem is ~0.1% of the output so bf16 is harmless
            y3_sb = cpool.tile([P, NTB, DIM], mybir.dt.bfloat16)
            # all 8 pass-B gen tiles stay resident (2 MiB fp8) so pass B can
            # run tb-outer and finish whole token blocks early
            g3_sb = cpool.tile([P, R_LOC, KC_H, 512], f8)

            # All resident loads go on the ACT HWDGE ring, ordered by first
            # use (hgT inputs first); the SP ring is reserved for the g2p
            # stream so the first gen tile lands within ~3us.
            mtt_r = mtt_d.ap().rearrange("(kc p) t -> p kc t", p=P)
            for kc in range(KC_DM):
                nc.sync.dma_start(out=mtt_sb[:, kc, :], in_=mtt_r[:, kc, :])
            nc.scalar.dma_start(out=gw1t_sb[:], in_=gw1t_d.ap().rearrange("(kc p) h -> p kc h", p=P))
            nc.scalar.dma_start(out=gb1_sb[:], in_=gb1_d.ap().rearrange("(hc p) -> p hc", p=P))
            # First gen-weight tile DMAs are hoisted ahead of the resident
            # bulk (SP ring) so the gen stream can start as soon as hgT is
            # ready; x is interleaved right behind since pass A's reduce
            # consumes it within a few us of the first gen matmul.
            # (Loop-bench mode can't hoist: the slots would never recycle.)
            bench_iters = int(_os.environ.get("KERNEL_BENCH_ITERS", "1"))
            N_HOIST = 4 if bench_iters == 1 else 0
            gt_hoist = []
            x_r = x_d.ap().rearrange("(tb p) d -> p tb d", p=P)
            for rb in range(N_HOIST):
                gth = g2pool.tile([P, KC_H, 512], f8, tag="gt")
                nc.sync.dma_start(out=gth[:], in_=g2p_d[rb])
                gt_hoist.append(gth)
                if rb == 0:
                    for tb in range(3):
                        nc.sync.dma_start(out=x_sb[:, tb, :], in_=x_r[:, tb, :])
                elif rb == 1:
                    for tb in range(3, NTB):
                        nc.sync.dma_start(out=x_sb[:, tb, :], in_=x_r[:, tb, :])
            if N_HOIST == 0:
                for tb in range(NTB):
                    nc.sync.dma_start(out=x_sb[:, tb, :], in_=x_r[:, tb, :])
            # Everything needed only after pass A rides the idle GPSIMD
            # SWDGE ring.
            nc.gpsimd.dma_start(out=gate_sb[:], in_=gate_d.ap().to_broadcast((P, 1)))
            xt_r = xt_d.ap().rearrange("(kc p) t -> p kc t", p=P)
            for kc in range(KC_D):
                nc.gpsimd.dma_start(out=xt_sb[:, kc, :], in_=xt_r[:, kc, :])
            w1t_r = w1t_d.ap().rearrange("(kc p) h -> p kc h", p=P)
            w2t_r = w2t_d.ap().rearrange("(kc p) h -> p kc h", p=P)
            w3t_r = w3t_d.ap().rearrange("(kc p) d -> p kc d", p=P)
            for kc in range(KC_D):
                nc.gpsimd.dma_start(out=w1t_sb[:, kc, :], in_=w1t_r[:, kc, :])
                nc.gpsimd.dma_start(out=w2t_sb[:, kc, :], in_=w2t_r[:, kc, :])
            nc.gpsimd.dma_start(out=bb1_sb[:], in_=bb1_d.ap().rearrange("(hc p) -> p hc", p=P))
            nc.gpsimd.dma_start(out=bb2_sb[:], in_=bb2_d.ap().rearrange("(hc p) -> p hc", p=P))
            if not gen_bias_zero:
                nc.gpsimd.dma_start(out=b1c_sb[:], in_=b1c_d.ap().rearrange("kc p r -> p kc r"))
                nc.gpsimd.dma_start(out=b2c_sb[:], in_=b2c_d.ap().rearrange("kc p r -> p kc r"))
                nc.gpsimd.dma_start(out=b3ct_sb[:], in_=b3ct_d.ap())
            if not b3_zero:
                nc.gpsimd.dma_start(out=b3_sb[:1, :], in_=b3_d.ap().unsqueeze(0))
            for kc in range(KC_HID):
                nc.gpsimd.dma_start(out=w3t_sb[:, kc, :], in_=w3t_r[:, kc, :])

            if not gen_bias_zero:
                make_identity(nc, ident_sb[:])
            if not b3_zero:
                nc.vector.memset(ones1f_sb[:1, :], 1.0)
                nc.scalar.activation(ones1_sb[:1, :], ones1f_sb[:1, :], Act.Copy,
                                     bias=0.0, scale=1.0)
                # b3 / 8 (each core contributes an eighth; ReduceScatter sums)
                nc.scalar.activation(b3r_sb[:1, :], b3_sb[:1, :], Act.Copy,
                                     bias=0.0, scale=0.125)

            # ---- hgT = silu(gen_w1 @ m_tok + gen_b1), layout [h, tokens] ----
            with tc.tile_pool(name="hgps", bufs=2, space="PSUM") as hgps:
                for hb in range(KC_H):
                    ps = hgps.tile([P, NTOK], f32)
                    for tn in range(2):
                        for kc in range(KC_DM):
                            nc.tensor.matmul(
                                ps[:, tn * 512:(tn + 1) * 512],
                                gw1t_sb[:, kc, hb * P:(hb + 1) * P],
                                mtt_sb[:, kc, tn * 512:(tn + 1) * 512],
                                start=(kc == 0), stop=(kc == KC_DM - 1),
                            )
                    if _exact_silu:
                        # silu(z) = z * sigmoid(z), z = ps + gen_b1 (matches
                        # jax exactly; CoreSim has no Silu LUT); DVE writes a
                        # f32 scratch, ACT converts to the fp8 gen operand
                        sg = spool.tile([P, NTOK], f32, tag="hg_sg")
                        nc.scalar.activation(sg[:], ps[:], Act.Sigmoid,
                                             bias=gb1_sb[:, hb:hb + 1], scale=1.0)
                        sf = spool.tile([P, NTOK], f32, tag="hg_sf")
                        nc.vector.scalar_tensor_tensor(
                            out=sf[:], in0=ps[:],
                            scalar=gb1_sb[:, hb:hb + 1], in1=sg[:],
                            op0=Alu.add, op1=Alu.mult,
                        )
                        nc.scalar.activation(hgt_sb[:, hb, :], sf[:], Act.Copy,
                                             bias=0.0, scale=1.0)
                    else:
                        # HW Silu LUT — split per token-half so the last
                        # chunk's first half (which gates pass A's first DR
                        # matmul) is ready one half-silu earlier
                        for tn in range(2):
                            tsl = slice(tn * 512, (tn + 1) * 512)
                            nc.scalar.activation(hgt_sb[:, hb, tsl], ps[:, tsl],
                                                 Act.Silu,
                                                 bias=gb1_sb[:, hb:hb + 1],
                                                 scale=1.0)

            cc_in = dpool.tile([NTOK, DIM], f32)
            cc_out = dpool.tile([P, DIM], f32)

            with (
                tc.tile_pool(name="genps", bufs=5, space="PSUM") as genps,
                tc.tile_pool(name="trps", bufs=1, space="PSUM") as trps,
                tc.tile_pool(name="ybps", bufs=2, space="PSUM") as ybps,
            ):
                def main_body():
                    # -- pass A: generate W1_m / W2_m row-blocks, reduce vs x --
                    for rb in range(2 * R_LOC):  # 0..7 -> W1_m r, 8..15 -> W2_m r
                        mat, r = divmod(rb, R_LOC)
                        if rb < N_HOIST:
                            gt = gt_hoist[rb]
                        else:
                            gt = g2pool.tile([P, KC_H, 512], f8, tag="gt")
                            nc.sync.dma_start(out=gt[:], in_=g2p_d[rb])
                        u_sb = u1_sb if mat == 0 else u2_sb
                        for tb in range(NTB):
                            wps = genps.tile([P, 512], f32, tag="wps")
                            if _no_dr:
                                for hc in range(KC_H):
                                    nc.tensor.matmul(
                                        wps[:],
                                        hgt_sb[:, hc, tb * P:(tb + 1) * P],
                                        gt[:, hc, :],
                                        start=(hc == 0), stop=(hc == KC_H - 1),
                                    )
                            else:
                                for pc in range(KC_H // 2):
                                    nc.tensor.matmul(
                                        wps[:],
                                        hgt_sb[:, 2 * pc:2 * pc + 2, tb * P:(tb + 1) * P],
                                        gt[:, 2 * pc:2 * pc + 2, :],
                                        start=(pc == 0), stop=(pc == KC_H // 2 - 1),
                                        perf_mode=DR,
                                    )
                            scr = spool.tile([P, 512], f32, tag="scr")
                            # u[t, r] = sum_d w[t, (r, d)] * x[t, d]  (bias added
                            # later; tensor_tensor_reduce would fuse it but
                            # crashes on HW)
                            nc.vector.scalar_tensor_tensor(
                                out=scr[:], in0=wps[:], scalar=1.0,
                                in1=x_sb[:, tb, :],
                                op0=Alu.mult, op1=Alu.mult,
                                accum_out=u_sb[:, tb, r:r + 1],
                            )

                    # pass-B gen tiles stream behind the pass-A tiles on the
                    # same ring; they are resident by the time pass B starts
                    for r3 in range(R_LOC):
                        nc.sync.dma_start(out=g3_sb[:, r3],
                                          in_=g2p_d[2 * R_LOC + r3])

                    # -- base path: hT = silu(W1 x + b1) * (W2 x + b2) --
                    # (emitted after pass A so the PE doesn't stall on xt/w1t
                    # loads before reaching the gen stream; psum tiles share
                    # the wps rotation)
                    for hc in range(KC_HID):
                        for tn in range(2):
                            tsl = slice(tn * 512, (tn + 1) * 512)
                            ps1 = genps.tile([P, 512], f32, tag="wps")
                            ps2 = genps.tile([P, 512], f32, tag="wps")
                            for kc in range(KC_D):
                                nc.tensor.matmul(
                                    ps1[:], w1t_sb[:, kc, hc * P:(hc + 1) * P],
                                    xt_sb[:, kc, tsl],
                                    start=(kc == 0), stop=(kc == KC_D - 1),
                                )
                                nc.tensor.matmul(
                                    ps2[:], w2t_sb[:, kc, hc * P:(hc + 1) * P],
                                    xt_sb[:, kc, tsl],
                                    start=(kc == 0), stop=(kc == KC_D - 1),
                                )
                            s1 = spool.tile([P, 512], f32, tag="scr")
                            if _exact_silu:
                                sg1 = spool.tile([P, 512], f32, tag="scr")
                                nc.scalar.activation(sg1[:], ps1[:], Act.Sigmoid,
                                                     bias=bb1_sb[:, hc:hc + 1],
                                                     scale=1.0)
                                nc.vector.scalar_tensor_tensor(
                                    out=s1[:], in0=ps1[:],
                                    scalar=bb1_sb[:, hc:hc + 1], in1=sg1[:],
                                    op0=Alu.add, op1=Alu.mult,
                                )
                            else:
                                nc.scalar.activation(s1[:], ps1[:], Act.Silu,
                                                     bias=bb1_sb[:, hc:hc + 1],
                                                     scale=1.0)
                            nc.vector.scalar_tensor_tensor(
                                out=ht_sb[:, hc, tsl], in0=ps2[:],
                                scalar=bb2_sb[:, hc:hc + 1], in1=s1[:],
                                op0=Alu.add, op1=Alu.mult,
                            )

                    # -- u-path bias terms: ub{1,2}[t, r] = x[t] @ gen_b2_blk --
                    if not gen_bias_zero:
                        for tb in range(NTB):
                            uc1 = genps.tile([P, R_LOC], f32, tag="wps")
                            uc2 = genps.tile([P, R_LOC], f32, tag="wps")
                            for kc in range(KC_D):
                                nc.tensor.matmul(
                                    uc1[:], xt_sb[:, kc, tb * P:(tb + 1) * P],
                                    b1c_sb[:, kc, :],
                                    start=(kc == 0), stop=(kc == KC_D - 1))
                                nc.tensor.matmul(
                                    uc2[:], xt_sb[:, kc, tb * P:(tb + 1) * P],
                                    b2c_sb[:, kc, :],
                                    start=(kc == 0), stop=(kc == KC_D - 1))
                            nc.scalar.activation(ub1_sb[:, tb, :], uc1[:], Act.Copy,
                                                 bias=0.0, scale=1.0)
                            nc.scalar.activation(ub2_sb[:, tb, :], uc2[:], Act.Copy,
                                                 bias=0.0, scale=1.0)

                    # -- h_mem = g * silu(u1 + ub1) * (u2 + ub2), + transpose --
                    for tb in range(NTB):
                        if not gen_bias_zero:
                            nc.vector.tensor_add(u1_sb[:, tb, :], u1_sb[:, tb, :],
                                                 ub1_sb[:, tb, :])
                            nc.vector.tensor_add(u2_sb[:, tb, :], u2_sb[:, tb, :],
                                                 ub2_sb[:, tb, :])
                        s = spool.tile([P, R_LOC], f32, tag="hm_s")
                        if _exact_silu:
                            nc.scalar.activation(s[:], u1_sb[:, tb, :], Act.Sigmoid)
                            nc.vector.tensor_mul(s[:], s[:], u1_sb[:, tb, :])
                        else:
                            nc.scalar.activation(s[:], u1_sb[:, tb, :], Act.Silu)
                        nc.vector.tensor_mul(s[:], s[:], u2_sb[:, tb, :])
                        # gate_sb = sigmoid(mem_gate)/4096 (host-computed);
                        # the /4096 cancels the gen_w2 fp8 pre-scale on the
                        # pass-B psum tiles
                        nc.vector.tensor_scalar_mul(hmg_sb[:, tb, :], s[:],
                                                    gate_sb[:, :1])
                        if not gen_bias_zero:
                            tp = trps.tile([R_LOC, P], f32, tag="tp")
                            nc.tensor.transpose(tp[:], hmg_sb[:, tb, :], ident_sb[:])
                            nc.scalar.activation(hmt_sb[:, tb, :], tp[:], Act.Copy,
                                                 bias=0.0, scale=1.0)

                    def emit_final(tb):
                        # yb = W3 h (+ b3/8 + y_mem bias corr); out = yb + y3acc
                        yb = ybps.tile([P, DIM], f32, tag="yb")
                        n_mm = KC_HID + (not b3_zero) + (not gen_bias_zero)
                        mm_i = 0
                        for hc in range(KC_HID):
                            mm_i += 1
                            nc.tensor.matmul(yb[:], ht_sb[:, hc, tb * P:(tb + 1) * P],
                                             w3t_sb[:, hc, :],
                                             start=(hc == 0), stop=(mm_i == n_mm))
                        if not b3_zero:
                            mm_i += 1
                            nc.tensor.matmul(yb[:], ones1_sb[:1, :], b3r_sb[:1, :],
                                             start=False, stop=(mm_i == n_mm))
                        if not gen_bias_zero:
                            mm_i += 1
                            nc.tensor.matmul(yb[:], hmt_sb[:, tb, :], b3ct_sb[:],
                                             start=False, stop=(mm_i == n_mm))
                        out_t = opool.tile([P, DIM], f32, tag="out_t")
                        nc.vector.tensor_add(out_t[:], yb[:], y3_sb[:, tb, :])
                        nc.sync.dma_start(out=cc_in[tb * P:(tb + 1) * P, :],
                                          in_=out_t[:])

                    # -- pass B: tb-outer so each token block (and its RS
                    #    chunk) completes early.  Per (tb, r): ACT drains the
                    #    generated psum tile scaled by hmg[t, r] (per-token
                    #    scale) to bf16; DVE accumulates y3 with 2x-rate
                    #    16-bit adds.  r == 0 initializes y3 via the drain
                    #    itself --
                    for tb in range(NTB):
                        for r in range(R_LOC):
                            wps = genps.tile([P, 512], f32, tag="wps")
                            if _no_dr:
                                for hc in range(KC_H):
                                    nc.tensor.matmul(
                                        wps[:],
                                        hgt_sb[:, hc, tb * P:(tb + 1) * P],
                                        g3_sb[:, r, hc, :],
                                        start=(hc == 0), stop=(hc == KC_H - 1),
                                    )
                            else:
                                for pc in range(KC_H // 2):
                                    nc.tensor.matmul(
                                        wps[:],
                                        hgt_sb[:, 2 * pc:2 * pc + 2, tb * P:(tb + 1) * P],
                                        g3_sb[:, r, 2 * pc:2 * pc + 2, :],
                                        start=(pc == 0), stop=(pc == KC_H // 2 - 1),
                                        perf_mode=DR,
                                    )
                            if r == 0:
                                nc.scalar.activation(
                                    y3_sb[:, tb, :], wps[:], Act.Copy,
                                    bias=0.0, scale=hmg_sb[:, tb, r:r + 1])
                            else:
                                s16 = spool.tile([P, 512], mybir.dt.bfloat16,
                                                 tag="s16")
                                nc.scalar.activation(
                                    s16[:], wps[:], Act.Copy,
                                    bias=0.0, scale=hmg_sb[:, tb, r:r + 1])
                                nc.vector.tensor_add(
                                    y3_sb[:, tb, :], y3_sb[:, tb, :], s16[:])
                        emit_final(tb)
                        if not _skip_rs and _rs_chunks > 1:
                            nc.gpsimd.collective_compute(
                                "ReduceScatter",
                                mybir.AluOpType.add,
                                replica_groups=[list(range(NCORES))],
                                ins=[cc_in[tb * P:(tb + 1) * P, :].opt()],
                                outs=[cc_out[tb * RS_O:(tb + 1) * RS_O, :].opt()],
                            )

                if bench_iters > 1:
                    with tc.For_i(0, bench_iters, 1):
                        main_body()
                else:
                    main_body()

                if _skip_rs:
                    nc.sync.dma_start(out=y_out_d.ap(), in_=cc_in[:])
                else:
                    if _rs_chunks == 1:
                        nc.gpsimd.collective_compute(
                            "ReduceScatter",
                            mybir.AluOpType.add,
                            replica_groups=[list(range(NCORES))],
                            ins=[cc_in.opt()],
                            outs=[cc_out.opt()],
                        )
                    nc.sync.dma_start(out=y_out_d.ap(), in_=cc_out[:])

    nc.compile()
    return nc


def _prep_inputs(x, m_tok, W1, W2, W3, b1, b2, b3, gen_w1, gen_b1, gen_w2,
                 gen_b2, mem_gate):
    """Shard + relayout full inputs into 8 per-core input maps (numpy only)."""
    import ml_dtypes
    f4 = np.float32
    f8np = ml_dtypes.float8_e4m3
    GS = np.float32(4096.0)  # gen_w2 fp8 pre-scale (power of 2: exact inverse)
    x2d = np.ascontiguousarray(x.reshape(NTOK, DIM), dtype=f4)
    xt = np.ascontiguousarray(x2d.T)
    xs = x2d * (1.0 / GS)  # pass-A reduce operand; cancels the gen pre-scale
    mtt = np.ascontiguousarray(m_tok.reshape(NTOK, D_M).T, dtype=f4)
    gw1t = np.ascontiguousarray(np.asarray(gen_w1, f4).T)
    gen_w2 = np.asarray(gen_w2, f4)
    gen_b2 = np.asarray(gen_b2, f4)
    # gate carries sigmoid (host-computed scalar) and the pass-B descale
    gate = (1.0 / (1.0 + np.exp(-np.asarray(mem_gate, np.float64))) / f4(GS))
    gate = np.asarray(gate, f4).reshape(1, 1)
    W1 = np.asarray(W1, f4)
    W2 = np.asarray(W2, f4)
    W3 = np.asarray(W3, f4)

    # W3_m block of gen_w2 reordered r-major: [R, DIM, GEN_HIDDEN]
    g3_rmaj = gen_w2[2 * RD:].reshape(DIM, R, GEN_HIDDEN).transpose(1, 0, 2)
    b3_rmaj = gen_b2[2 * RD:].reshape(DIM, R)  # [d, r]

    in_maps = []
    for c in range(NCORES):
        rsl = slice(c * R_LOC * DIM, (c + 1) * R_LOC * DIM)
        g1 = gen_w2[rsl]
        g2 = gen_w2[RD + c * R_LOC * DIM: RD + (c + 1) * R_LOC * DIM]
        g3 = g3_rmaj[c * R_LOC:(c + 1) * R_LOC].reshape(R_LOC * DIM, GEN_HIDDEN)
        gcat = np.concatenate([g1, g2, g3], axis=0)  # [12288 rows, 512 h]
        # pack to [rb, p, hc, col]: gpack[rb, p, hc, col] = gcat[rb*512+col, hc*128+p]
        # scaled x4096 into fp8 e4m3 (rms ~3.6, |max| ~20 — mid-range)
        gpack = np.ascontiguousarray(
            (gcat * GS).reshape(N_RB, 512, KC_H, P).transpose(0, 3, 2, 1)
        ).astype(f8np)
        b1c = np.ascontiguousarray(
            gen_b2[rsl].reshape(R_LOC, DIM).T.reshape(KC_D, P, R_LOC))
        b2c = np.ascontiguousarray(
            gen_b2[RD + c * R_LOC * DIM: RD + (c + 1) * R_LOC * DIM]
            .reshape(R_LOC, DIM).T.reshape(KC_D, P, R_LOC))
        # hmg carries 1/4096 (folded into gate); compensate the bias-corr
        # matmul operand
        b3ct = np.ascontiguousarray(b3_rmaj[:, c * R_LOC:(c + 1) * R_LOC].T) * GS
        hsl = slice(c * HID_LOC, (c + 1) * HID_LOC)
        in_maps.append({
            "x": xs,
            "xt": xt,
            "mtt": mtt,
            "gw1t": gw1t,
            "gb1": np.asarray(gen_b1, f4),
            "g2p": gpack,
            "b1c": b1c,
            "b2c": b2c,
            "b3ct": b3ct,
            "w1t": np.ascontiguousarray(W1[hsl].T),
            "w2t": np.ascontiguousarray(W2[hsl].T),
            "w3t": np.ascontiguousarray(W3[:, hsl].T),
            "bb1": np.asarray(b1, f4)[hsl],
            "bb2": np.asarray(b2, f4)[hsl],
            "b3": np.asarray(b3, f4),
            "gate": gate,
        })
    return in_maps


def kernel(**inputs):
    from concourse.bass_utils import run_bass_kernel_spmd

    gen_bias_zero = not np.any(np.asarray(inputs["gen_b2"]))
    b3_zero = not np.any(np.asarray(inputs["b3"]))
    key = ("nc", gen_bias_zero, b3_zero)
    if key not in _CACHE:
        _CACHE[key] = _build_program(gen_bias_zero=gen_bias_zero,
                                     b3_zero=b3_zero)
    nc = _CACHE[key]

    in_maps = _prep_inputs(**{k: np.asarray(v) for k, v in inputs.items()})
    res = run_bass_kernel_spmd(nc, in_maps, core_ids=list(range(NCORES)))
    import os as _os
    if bool(int(_os.environ.get("KERNEL_SKIP_RS", "0"))):
        y = sum(res.results[c]["y_out"] for c in range(NCORES))
    else:
        Y = np.stack([res.results[c]["y_out"] for c in range(NCORES)])
        if int(_os.environ.get("KERNEL_RS_CHUNKS", "8")) == 1:
            y = Y.reshape(NTOK, DIM)
        else:
            # chunked RS: core c's rows [16 tb : 16 tb + 16] hold tokens
            # 128 tb + 16 c + [0, 16)
            y = Y.reshape(NCORES, NTB, P // NCORES, DIM).transpose(
                1, 0, 2, 3).reshape(NTOK, DIM)
    return y.reshape(B, T, DIM).astype(np.float32)



# revision 32
# speedup vs baseline: 1.4965x; 1.4965x over previous
"""Trainium2 Bass kernel for FFNDynamicMemories (SwiGLU FFN + per-token
hypernetwork-generated low-rank memory path).

Sharding strategy (8 cores):
  - The dominant cost is gen_w2 [3*R*DIM=98304, GEN_HIDDEN=512] (192 MiB fp32):
    it is sharded across cores by *generated-weight rows*: core c owns
    r in [8c, 8c+8) of W1_m and W2_m rows, plus the matching r-slice of W3_m
    (reordered r-major on the host so each 512-row block is one r).
    Each core therefore reads only 1/8 of gen_w2.
  - hg = silu(gen_w1 @ m_tok) is computed by every core for all tokens (tiny).
  - The base SwiGLU FFN is sharded by hidden dim (256 of 2048 per core).
  - Each core produces a full-shape partial y; one on-device ReduceScatter
    (over the token axis) combines them; the host just concatenates the
    8 [128, 512] shards.

Compute mapping per core:
  - Generation matmuls run in fp8e4 (e4m3) with MatmulPerfMode.DoubleRow
    (2 K-rows per free-dim slot, ~1.4x the bf16/fp32r PE rate at FD=512,
    and 4x less HBM traffic for the gen_w2 stream).  gen_w2 is host-scaled
    by 4096 so its ~1e-3-rms values land mid-range in e4m3; the u-path
    compensates with x/4096 (host) and the y_mem path with
    sigmoid(gate)/4096 folded into the per-token h_mem scalars.  The
    mem-path output is ~0.1% of the final scale, so fp8's ~4% error there
    contributes ~3e-4 relative — same order as the fp32r baseline.
    w_tile[t_block 128, 512 rows] = hgT_pair.T @ gen_w2T_pair, accumulated
    over 2 h-pair-chunks (each K=256) in PSUM.
  - The per-token "apply" (u1/u2 row-dot with x, y_mem accumulation over r)
    uses fused DVE ops reading the generated tile straight from PSUM:
      u[t, r]   = tensor_tensor_reduce(w_psum * x, +, init=x@gen_b2_slice)
      y3acc[t]  += scalar_tensor_tensor(w_psum * h_mem[t, r]) (in-place add)
  - gen_w2 row-blocks stream in the OUTER loop (tokens inner) so the big
    tensor is read from HBM exactly once.
"""

import sys

sys.path.insert(0, "/opt/trn_rl_repo")

import numpy as np

# Problem constants (hardcoded per the harness contract).
B, T, DIM, HIDDEN, D_M, R = 2, 512, 512, 2048, 256, 64
GEN_HIDDEN = 2 * D_M  # 512
NTOK = B * T  # 1024
RD = R * DIM  # 32768
NCORES = 8
R_LOC = R // NCORES  # 8 r values per core
HID_LOC = HIDDEN // NCORES  # 256 hidden per core
P = 128
NTB = NTOK // P  # 8 token blocks
N_RB = 3 * R_LOC  # 24 row-blocks of 512 generated rows per core
KC_H = GEN_HIDDEN // P  # 4 h-chunks
KC_D = DIM // P  # 4 d-chunks
KC_DM = D_M // P  # 2 d_m chunks
KC_HID = HID_LOC // P  # 2 hidden chunks

_CACHE = {}


def _build_program(gen_bias_zero=False, b3_zero=False):
    """gen_bias_zero/b3_zero: value-specialization flags (checked by kernel()
    against the actual inputs; a mismatch rebuilds the program). When gen_b2
    is all-zero the u/y_mem bias-correction matmuls, the h_mem transpose and
    the y3corr matmul are elided; when b3 is all-zero its K=1 row matmul is."""
    import concourse.bass as bass
    import concourse.mybir as mybir
    import concourse.tile as tile
    from concourse import bacc
    from concourse.masks import make_identity

    f32 = mybir.dt.float32
    f32r = mybir.dt.float32r
    f8 = mybir.dt.float8e4
    DR = mybir.MatmulPerfMode.DoubleRow
    Act = mybir.ActivationFunctionType
    Alu = mybir.AluOpType

    nc = bacc.Bacc("TRN2", target_bir_lowering=False, debug=False,
                   num_devices=NCORES)

    # ---- DRAM parameters (per-core shapes) ----
    bf16 = mybir.dt.bfloat16
    x_d = nc.dram_tensor("x", [NTOK, DIM], bf16, kind="ExternalInput")
    xt_d = nc.dram_tensor("xt", [DIM, NTOK], f32r, kind="ExternalInput")
    mtt_d = nc.dram_tensor("mtt", [D_M, NTOK], bf16, kind="ExternalInput")
    gw1t_d = nc.dram_tensor("gw1t", [D_M, GEN_HIDDEN], bf16, kind="ExternalInput")
    gb1_d = nc.dram_tensor("gb1", [GEN_HIDDEN], f32, kind="ExternalInput")
    # packed gen_w2 shard: [rb, p(h within chunk), hc, 512 rows], fp8 x4096
    g2p_d = nc.dram_tensor("g2p", [N_RB, P, KC_H, 512], f8, kind="ExternalInput")
    b1c_d = nc.dram_tensor("b1c", [KC_D, P, R_LOC], f32r, kind="ExternalInput")
    b2c_d = nc.dram_tensor("b2c", [KC_D, P, R_LOC], f32r, kind="ExternalInput")
    b3ct_d = nc.dram_tensor("b3ct", [R_LOC, DIM], f32r, kind="ExternalInput")
    w1t_d = nc.dram_tensor("w1t", [DIM, HID_LOC], f32r, kind="ExternalInput")
    w2t_d = nc.dram_tensor("w2t", [DIM, HID_LOC], f32r, kind="ExternalInput")
    w3t_d = nc.dram_tensor("w3t", [HID_LOC, DIM], f32r, kind="ExternalInput")
    bb1_d = nc.dram_tensor("bb1", [HID_LOC], f32, kind="ExternalInput")
    bb2_d = nc.dram_tensor("bb2", [HID_LOC], f32, kind="ExternalInput")
    b3_d = nc.dram_tensor("b3", [DIM], f32, kind="ExternalInput")
    gate_d = nc.dram_tensor("gate", [1, 1], f32, kind="ExternalInput")
    import os as _os
    _skip_rs = bool(int(_os.environ.get("KERNEL_SKIP_RS", "0")))
    _exact_silu = bool(int(_os.environ.get("KERNEL_EXACT_SILU", "0")))
    # ReduceScatter chunking: 8 = one collective per finished token block
    # (overlaps all but the last chunk with pass-B compute); 1 = single RS
    _rs_chunks = int(_os.environ.get("KERNEL_RS_CHUNKS", "8"))
    RS_O = P // NCORES  # 16 output rows per core per RS chunk
    # A/B probe: disable DoubleRow (plain fp8 matmuls, 4 K-chunks) to see
    # whether the real PE rate binds the measured time
    _no_dr = bool(int(_os.environ.get("KERNEL_NO_DR", "0")))
    if _skip_rs:
        y_out_d = nc.dram_tensor("y_out", [NTOK, DIM], f32, kind="ExternalOutput")
    else:
        y_out_d = nc.dram_tensor("y_out", [P, DIM], f32, kind="ExternalOutput")

    with tile.TileContext(nc) as tc:
        with (
            tc.tile_pool(name="const", bufs=1) as cpool,
            tc.tile_pool(name="g2s", bufs=4) as g2pool,
            tc.tile_pool(name="scr", bufs=3) as spool,
            tc.tile_pool(name="outp", bufs=3) as opool,
            tc.tile_pool(name="dram", bufs=1, space="DRAM") as dpool,
        ):
            # ---- resident SBUF tiles ----
            x_sb = cpool.tile([P, NTB, DIM], bf16)
            xt_sb = cpool.tile([P, KC_D, NTOK], f32r)
            mtt_sb = cpool.tile([P, KC_DM, NTOK], bf16)
            gw1t_sb = cpool.tile([P, KC_DM, GEN_HIDDEN], bf16)
            gb1_sb = cpool.tile([P, KC_H], f32)
            w1t_sb = cpool.tile([P, KC_D, HID_LOC], f32r)
            w2t_sb = cpool.tile([P, KC_D, HID_LOC], f32r)
            w3t_sb = cpool.tile([P, KC_HID, DIM], f32r)
            bb1_sb = cpool.tile([P, KC_HID], f32)
            bb2_sb = cpool.tile([P, KC_HID], f32)
            b3_sb = cpool.tile([1, DIM], f32)
            b3r_sb = cpool.tile([1, DIM], f32r)
            b1c_sb = cpool.tile([P, KC_D, R_LOC], f32r)
            b2c_sb = cpool.tile([P, KC_D, R_LOC], f32r)
            b3ct_sb = cpool.tile([R_LOC, DIM], f32r)
            gate_sb = cpool.tile([P, 1], f32)
            ident_sb = cpool.tile([P, P], f32)
            ones1f_sb = cpool.tile([1, P], f32)
            ones1_sb = cpool.tile([1, P], f32r)
            hgt_sb = cpool.tile([P, KC_H, NTOK], f8)
            ht_sb = cpool.tile([P, KC_HID, NTOK], f32r)
            ub1_sb = cpool.tile([P, NTB, R_LOC], f32)
            ub2_sb = cpool.tile([P, NTB, R_LOC], f32)
            u1_sb = cpool.tile([P, NTB, R_LOC], f32)
            u2_sb = cpool.tile([P, NTB, R_LOC], f32)
            hmg_sb = cpool.tile([P, NTB, R_LOC], f32)
            hmt_sb = cpool.tile([R_LOC, NTB, P], f32r)
            # y3 accumulates in bf16: the ACT engine drains each pass-B psum
            # tile (scaled by hmg) to bf16, and DVE adds run at the 2x
            # 16-bit rate; y_mem is ~0.1% of the output so bf16 is harmless
            y3_sb = cpool.tile([P, NTB, DIM], mybir.dt.bfloat16)
            # all 8 pass-B gen tiles stay resident (2 MiB fp8) so pass B can
            # run tb-outer and finish whole token blocks early
            g3_sb = cpool.tile([P, R_LOC, KC_H, 512], f8)

            # All resident loads go on the ACT HWDGE ring, ordered by first
            # use (hgT inputs first); the SP ring is reserved for the g2p
            # stream so the first gen tile lands within ~3us.
            mtt_r = mtt_d.ap().rearrange("(kc p) t -> p kc t", p=P)
            for kc in range(KC_DM):
                nc.sync.dma_start(out=mtt_sb[:, kc, :], in_=mtt_r[:, kc, :])
            nc.scalar.dma_start(out=gw1t_sb[:], in_=gw1t_d.ap().rearrange("(kc p) h -> p kc h", p=P))
            nc.scalar.dma_start(out=gb1_sb[:], in_=gb1_d.ap().rearrange("(hc p) -> p hc", p=P))
            # First gen-weight tile DMAs are hoisted ahead of the resident
            # bulk (SP ring) so the gen stream can start as soon as hgT is
            # ready; x is interleaved right behind since pass A's reduce
            # consumes it within a few us of the first gen matmul.
            # (Loop-bench mode can't hoist: the slots would never recycle.)
            bench_iters = int(_os.environ.get("KERNEL_BENCH_ITERS", "1"))
            N_HOIST = 4 if bench_iters == 1 else 0
            gt_hoist = []
            x_r = x_d.ap().rearrange("(tb p) d -> p tb d", p=P)
            for rb in range(N_HOIST):
                gth = g2pool.tile([P, KC_H, 512], f8, tag="gt")
                nc.sync.dma_start(out=gth[:], in_=g2p_d[rb])
                gt_hoist.append(gth)
                if rb == 0:
                    for tb in range(3):
                        nc.sync.dma_start(out=x_sb[:, tb, :], in_=x_r[:, tb, :])
                elif rb == 1:
                    for tb in range(3, NTB):
                        nc.sync.dma_start(out=x_sb[:, tb, :], in_=x_r[:, tb, :])
            if N_HOIST == 0:
                for tb in range(NTB):
                    nc.sync.dma_start(out=x_sb[:, tb, :], in_=x_r[:, tb, :])
            # Everything needed only after pass A rides the idle GPSIMD
            # SWDGE ring.
            nc.gpsimd.dma_start(out=gate_sb[:], in_=gate_d.ap().to_broadcast((P, 1)))
            xt_r = xt_d.ap().rearrange("(kc p) t -> p kc t", p=P)
            for kc in range(KC_D):
                nc.gpsimd.dma_start(out=xt_sb[:, kc, :], in_=xt_r[:, kc, :])
            w1t_r = w1t_d.ap().rearrange("(kc p) h -> p kc h", p=P)
            w2t_r = w2t_d.ap().rearrange("(kc p) h -> p kc h", p=P)
            w3t_r = w3t_d.ap().rearrange("(kc p) d -> p kc d", p=P)
            for kc in range(KC_D):
                nc.gpsimd.dma_start(out=w1t_sb[:, kc, :], in_=w1t_r[:, kc, :])
                nc.gpsimd.dma_start(out=w2t_sb[:, kc, :], in_=w2t_r[:, kc, :])
            nc.gpsimd.dma_start(out=bb1_sb[:], in_=bb1_d.ap().rearrange("(hc p) -> p hc", p=P))
            nc.gpsimd.dma_start(out=bb2_sb[:], in_=bb2_d.ap().rearrange("(hc p) -> p hc", p=P))
            if not gen_bias_zero:
                nc.gpsimd.dma_start(out=b1c_sb[:], in_=b1c_d.ap().rearrange("kc p r -> p kc r"))
                nc.gpsimd.dma_start(out=b2c_sb[:], in_=b2c_d.ap().rearrange("kc p r -> p kc r"))
                nc.gpsimd.dma_start(out=b3ct_sb[:], in_=b3ct_d.ap())
            if not b3_zero:
                nc.gpsimd.dma_start(out=b3_sb[:1, :], in_=b3_d.ap().unsqueeze(0))
            for kc in range(KC_HID):
                nc.gpsimd.dma_start(out=w3t_sb[:, kc, :], in_=w3t_r[:, kc, :])

            if not gen_bias_zero:
                make_identity(nc, ident_sb[:])
            if not b3_zero:
                nc.vector.memset(ones1f_sb[:1, :], 1.0)
                nc.scalar.activation(ones1_sb[:1, :], ones1f_sb[:1, :], Act.Copy,
                                     bias=0.0, scale=1.0)
                # b3 / 8 (each core contributes an eighth; ReduceScatter sums)
                nc.scalar.activation(b3r_sb[:1, :], b3_sb[:1, :], Act.Copy,
                                     bias=0.0, scale=0.125)

            # ---- hgT = silu(gen_w1 @ m_tok + gen_b1), layout [h, tokens] ----
            # all 4 h-chunk matmul groups run back-to-back on the PE (4 psum
            # bufs), then silus are emitted token-half-major so the halves
            # gating pass A's first DR matmuls complete first
            with tc.tile_pool(name="hgps", bufs=4, space="PSUM") as hgps:
                hg_ps = []
                for hb in range(KC_H):
                    ps = hgps.tile([P, NTOK], f32)
                    hg_ps.append(ps)
                    for tn in range(2):
                        for kc in range(KC_DM):
                            nc.tensor.matmul(
                                ps[:, tn * 512:(tn + 1) * 512],
                                gw1t_sb[:, kc, hb * P:(hb + 1) * P],
                                mtt_sb[:, kc, tn * 512:(tn + 1) * 512],
                                start=(kc == 0), stop=(kc == KC_DM - 1),
                            )
                if _exact_silu:
                    for hb in range(KC_H):
                        ps = hg_ps[hb]
                        # silu(z) = z * sigmoid(z), z = ps + gen_b1 (matches
                        # jax exactly; CoreSim has no Silu LUT); DVE writes a
                        # f32 scratch, ACT converts to the fp8 gen operand
                        sg = spool.tile([P, NTOK], f32, tag="hg_sg")
                        nc.scalar.activation(sg[:], ps[:], Act.Sigmoid,
                                             bias=gb1_sb[:, hb:hb + 1], scale=1.0)
                        sf = spool.tile([P, NTOK], f32, tag="hg_sf")
                        nc.vector.scalar_tensor_tensor(
                            out=sf[:], in0=ps[:],
                            scalar=gb1_sb[:, hb:hb + 1], in1=sg[:],
                            op0=Alu.add, op1=Alu.mult,
                        )
                        nc.scalar.activation(hgt_sb[:, hb, :], sf[:], Act.Copy,
                                             bias=0.0, scale=1.0)
                else:
                    for tn in range(2):
                        tsl = slice(tn * 512, (tn + 1) * 512)
                        for hb in range(KC_H):
                            nc.scalar.activation(hgt_sb[:, hb, tsl],
                                                 hg_ps[hb][:, tsl], Act.Silu,
                                                 bias=gb1_sb[:, hb:hb + 1],
                                                 scale=1.0)

            cc_in = dpool.tile([NTOK, DIM], f32)
            cc_out = dpool.tile([P, DIM], f32)

            with (
                tc.tile_pool(name="genps", bufs=5, space="PSUM") as genps,
                tc.tile_pool(name="trps", bufs=1, space="PSUM") as trps,
                tc.tile_pool(name="ybps", bufs=2, space="PSUM") as ybps,
            ):
                def emit_hmem(tb):
                    # h_mem[t, r] = gate * silu(u1) * u2 for one token block
                    s = spool.tile([P, R_LOC], f32, tag="hm_s")
                    if _exact_silu:
                        nc.scalar.activation(s[:], u1_sb[:, tb, :], Act.Sigmoid)
                        nc.vector.tensor_mul(s[:], s[:], u1_sb[:, tb, :])
                    else:
                        nc.scalar.activation(s[:], u1_sb[:, tb, :], Act.Silu)
                    nc.vector.tensor_mul(s[:], s[:], u2_sb[:, tb, :])
                    # gate_sb = sigmoid(mem_gate)/4096 (host-computed); the
                    # /4096 cancels the gen_w2 fp8 pre-scale on the pass-B
                    # psum tiles
                    nc.vector.tensor_scalar_mul(hmg_sb[:, tb, :], s[:],
                                                gate_sb[:, :1])

                def main_body():
                    # -- pass A: generate W1_m / W2_m row-blocks, reduce vs x --
                    for rb in range(2 * R_LOC):  # 0..7 -> W1_m r, 8..15 -> W2_m r
                        mat, r = divmod(rb, R_LOC)
                        if rb < N_HOIST:
                            gt = gt_hoist[rb]
                        else:
                            gt = g2pool.tile([P, KC_H, 512], f8, tag="gt")
                            nc.sync.dma_start(out=gt[:], in_=g2p_d[rb])
                        u_sb = u1_sb if mat == 0 else u2_sb
                        for tb in range(NTB):
                            wps = genps.tile([P, 512], f32, tag="wps")
                            if _no_dr:
                                for hc in range(KC_H):
                                    nc.tensor.matmul(
                                        wps[:],
                                        hgt_sb[:, hc, tb * P:(tb + 1) * P],
                                        gt[:, hc, :],
                                        start=(hc == 0), stop=(hc == KC_H - 1),
                                    )
                            else:
                                for pc in range(KC_H // 2):
                                    nc.tensor.matmul(
                                        wps[:],
                                        hgt_sb[:, 2 * pc:2 * pc + 2, tb * P:(tb + 1) * P],
                                        gt[:, 2 * pc:2 * pc + 2, :],
                                        start=(pc == 0), stop=(pc == KC_H // 2 - 1),
                                        perf_mode=DR,
                                    )
                            scr = spool.tile([P, 512], f32, tag="scr")
                            # u[t, r] = sum_d w[t, (r, d)] * x[t, d]  (bias added
                            # later; tensor_tensor_reduce would fuse it but
                            # crashes on HW)
                            nc.vector.scalar_tensor_tensor(
                                out=scr[:], in0=wps[:], scalar=1.0,
                                in1=x_sb[:, tb, :],
                                op0=Alu.mult, op1=Alu.mult,
                                accum_out=u_sb[:, tb, r:r + 1],
                            )
                            if rb == 2 * R_LOC - 1 and gen_bias_zero:
                                # u1/u2 for this tb are complete: emit its
                                # h_mem chain right away so pass B's drains
                                # aren't gated by a serial h_mem block
                                emit_hmem(tb)

                    # pass-B gen tiles stream behind the pass-A tiles on the
                    # same ring; they are resident by the time pass B starts
                    for r3 in range(R_LOC):
                        nc.sync.dma_start(out=g3_sb[:, r3],
                                          in_=g2p_d[2 * R_LOC + r3])

                    # -- base path: hT = silu(W1 x + b1) * (W2 x + b2) --
                    # (emitted after pass A so the PE doesn't stall on xt/w1t
                    # loads before reaching the gen stream; psum tiles share
                    # the wps rotation)
                    for hc in range(KC_HID):
                        for tn in range(2):
                            tsl = slice(tn * 512, (tn + 1) * 512)
                            ps1 = genps.tile([P, 512], f32, tag="wps")
                            ps2 = genps.tile([P, 512], f32, tag="wps")
                            for kc in range(KC_D):
                                nc.tensor.matmul(
                                    ps1[:], w1t_sb[:, kc, hc * P:(hc + 1) * P],
                                    xt_sb[:, kc, tsl],
                                    start=(kc == 0), stop=(kc == KC_D - 1),
                                )
                                nc.tensor.matmul(
                                    ps2[:], w2t_sb[:, kc, hc * P:(hc + 1) * P],
                                    xt_sb[:, kc, tsl],
                                    start=(kc == 0), stop=(kc == KC_D - 1),
                                )
                            s1 = spool.tile([P, 512], f32, tag="scr")
                            if _exact_silu:
                                sg1 = spool.tile([P, 512], f32, tag="scr")
                                nc.scalar.activation(sg1[:], ps1[:], Act.Sigmoid,
                                                     bias=bb1_sb[:, hc:hc + 1],
                                                     scale=1.0)
                                nc.vector.scalar_tensor_tensor(
                                    out=s1[:], in0=ps1[:],
                                    scalar=bb1_sb[:, hc:hc + 1], in1=sg1[:],
                                    op0=Alu.add, op1=Alu.mult,
                                )
                            else:
                                nc.scalar.activation(s1[:], ps1[:], Act.Silu,
                                                     bias=bb1_sb[:, hc:hc + 1],
                                                     scale=1.0)
                            nc.vector.scalar_tensor_tensor(
                                out=ht_sb[:, hc, tsl], in0=ps2[:],
                                scalar=bb2_sb[:, hc:hc + 1], in1=s1[:],
                                op0=Alu.add, op1=Alu.mult,
                            )

                    # -- u-path bias terms: ub{1,2}[t, r] = x[t] @ gen_b2_blk --
                    if not gen_bias_zero:
                        for tb in range(NTB):
                            uc1 = genps.tile([P, R_LOC], f32, tag="wps")
                            uc2 = genps.tile([P, R_LOC], f32, tag="wps")
                            for kc in range(KC_D):
                                nc.tensor.matmul(
                                    uc1[:], xt_sb[:, kc, tb * P:(tb + 1) * P],
                                    b1c_sb[:, kc, :],
                                    start=(kc == 0), stop=(kc == KC_D - 1))
                                nc.tensor.matmul(
                                    uc2[:], xt_sb[:, kc, tb * P:(tb + 1) * P],
                                    b2c_sb[:, kc, :],
                                    start=(kc == 0), stop=(kc == KC_D - 1))
                            nc.scalar.activation(ub1_sb[:, tb, :], uc1[:], Act.Copy,
                                                 bias=0.0, scale=1.0)
                            nc.scalar.activation(ub2_sb[:, tb, :], uc2[:], Act.Copy,
                                                 bias=0.0, scale=1.0)

                    # -- h_mem = g * silu(u1 + ub1) * (u2 + ub2), + transpose --
                    # (with zero gen bias this was already emitted inside
                    # pass A's last row-block, per token block)
                    if not gen_bias_zero:
                        for tb in range(NTB):
                            nc.vector.tensor_add(u1_sb[:, tb, :], u1_sb[:, tb, :],
                                                 ub1_sb[:, tb, :])
                            nc.vector.tensor_add(u2_sb[:, tb, :], u2_sb[:, tb, :],
                                                 ub2_sb[:, tb, :])
                            emit_hmem(tb)
                            tp = trps.tile([R_LOC, P], f32, tag="tp")
                            nc.tensor.transpose(tp[:], hmg_sb[:, tb, :], ident_sb[:])
                            nc.scalar.activation(hmt_sb[:, tb, :], tp[:], Act.Copy,
                                                 bias=0.0, scale=1.0)

                    def emit_final(tb):
                        # yb = W3 h (+ b3/8 + y_mem bias corr); out = yb + y3acc
                        yb = ybps.tile([P, DIM], f32, tag="yb")
                        n_mm = KC_HID + (not b3_zero) + (not gen_bias_zero)
                        mm_i = 0
                        for hc in range(KC_HID):
                            mm_i += 1
                            nc.tensor.matmul(yb[:], ht_sb[:, hc, tb * P:(tb + 1) * P],
                                             w3t_sb[:, hc, :],
                                             start=(hc == 0), stop=(mm_i == n_mm))
                        if not b3_zero:
                            mm_i += 1
                            nc.tensor.matmul(yb[:], ones1_sb[:1, :], b3r_sb[:1, :],
                                             start=False, stop=(mm_i == n_mm))
                        if not gen_bias_zero:
                            mm_i += 1
                            nc.tensor.matmul(yb[:], hmt_sb[:, tb, :], b3ct_sb[:],
                                             start=False, stop=(mm_i == n_mm))
                        out_t = opool.tile([P, DIM], f32, tag="out_t")
                        nc.vector.tensor_add(out_t[:], yb[:], y3_sb[:, tb, :])
                        nc.sync.dma_start(out=cc_in[tb * P:(tb + 1) * P, :],
                                          in_=out_t[:])

                    # -- pass B: tb-outer so each token block (and its RS
                    #    chunk) completes early.  Per (tb, r): ACT drains the
                    #    generated psum tile scaled by hmg[t, r] (per-token
                    #    scale) to bf16; DVE accumulates y3 with 2x-rate
                    #    16-bit adds.  r == 0 initializes y3 via the drain
                    #    itself --
                    for tb in range(NTB):
                        for r in range(R_LOC):
                            wps = genps.tile([P, 512], f32, tag="wps")
                            if _no_dr:
                                for hc in range(KC_H):
                                    nc.tensor.matmul(
                                        wps[:],
                                        hgt_sb[:, hc, tb * P:(tb + 1) * P],
                                        g3_sb[:, r, hc, :],
                                        start=(hc == 0), stop=(hc == KC_H - 1),
                                    )
                            else:
                                for pc in range(KC_H // 2):
                                    nc.tensor.matmul(
                                        wps[:],
                                        hgt_sb[:, 2 * pc:2 * pc + 2, tb * P:(tb + 1) * P],
                                        g3_sb[:, r, 2 * pc:2 * pc + 2, :],
                                        start=(pc == 0), stop=(pc == KC_H // 2 - 1),
                                        perf_mode=DR,
                                    )
                            if r == 0:
                                nc.scalar.activation(
                                    y3_sb[:, tb, :], wps[:], Act.Copy,
                                    bias=0.0, scale=hmg_sb[:, tb, r:r + 1])
                            else:
                                s16 = spool.tile([P, 512], mybir.dt.bfloat16,
                                                 tag="s16")
                                nc.scalar.activation(
                                    s16[:], wps[:], Act.Copy,
                                    bias=0.0, scale=hmg_sb[:, tb, r:r + 1])
                                nc.vector.tensor_add(
                                    y3_sb[:, tb, :], y3_sb[:, tb, :], s16[:])
                        emit_final(tb)
                        if not _skip_rs and _rs_chunks > 1:
                            nc.gpsimd.collective_compute(
                                "ReduceScatter",
                                mybir.AluOpType.add,
                                replica_groups=[list(range(NCORES))],
                                ins=[cc_in[tb * P:(tb + 1) * P, :].opt()],
                                outs=[cc_out[tb * RS_O:(tb + 1) * RS_O, :].opt()],
                            )

                if bench_iters > 1:
                    with tc.For_i(0, bench_iters, 1):
                        main_body()
                else:
                    main_body()

                if _skip_rs:
                    nc.sync.dma_start(out=y_out_d.ap(), in_=cc_in[:])
                else:
                    if _rs_chunks == 1:
                        nc.gpsimd.collective_compute(
                            "ReduceScatter",
                            mybir.AluOpType.add,
                            replica_groups=[list(range(NCORES))],
                            ins=[cc_in.opt()],
                            outs=[cc_out.opt()],
                        )
                    nc.sync.dma_start(out=y_out_d.ap(), in_=cc_out[:])

    nc.compile()
    return nc


def _prep_inputs(x, m_tok, W1, W2, W3, b1, b2, b3, gen_w1, gen_b1, gen_w2,
                 gen_b2, mem_gate):
    """Shard + relayout full inputs into 8 per-core input maps (numpy only)."""
    import ml_dtypes
    f4 = np.float32
    f8np = ml_dtypes.float8_e4m3
    GS = np.float32(4096.0)  # gen_w2 fp8 pre-scale (power of 2: exact inverse)
    bf = ml_dtypes.bfloat16
    x2d = np.ascontiguousarray(x.reshape(NTOK, DIM), dtype=f4)
    xt = np.ascontiguousarray(x2d.T)
    # pass-A reduce operand (bf16; cancels the gen pre-scale)
    xs = (x2d * (1.0 / GS)).astype(bf)
    mtt = np.ascontiguousarray(m_tok.reshape(NTOK, D_M).T.astype(bf))
    gw1t = np.ascontiguousarray(np.asarray(gen_w1, f4).T.astype(bf))
    gen_w2 = np.asarray(gen_w2, f4)
    gen_b2 = np.asarray(gen_b2, f4)
    # gate carries sigmoid (host-computed scalar) and the pass-B descale
    gate = (1.0 / (1.0 + np.exp(-np.asarray(mem_gate, np.float64))) / f4(GS))
    gate = np.asarray(gate, f4).reshape(1, 1)
    W1 = np.asarray(W1, f4)
    W2 = np.asarray(W2, f4)
    W3 = np.asarray(W3, f4)

    # W3_m block of gen_w2 reordered r-major: [R, DIM, GEN_HIDDEN]
    g3_rmaj = gen_w2[2 * RD:].reshape(DIM, R, GEN_HIDDEN).transpose(1, 0, 2)
    b3_rmaj = gen_b2[2 * RD:].reshape(DIM, R)  # [d, r]

    in_maps = []
    for c in range(NCORES):
        rsl = slice(c * R_LOC * DIM, (c + 1) * R_LOC * DIM)
        g1 = gen_w2[rsl]
        g2 = gen_w2[RD + c * R_LOC * DIM: RD + (c + 1) * R_LOC * DIM]
        g3 = g3_rmaj[c * R_LOC:(c + 1) * R_LOC].reshape(R_LOC * DIM, GEN_HIDDEN)
        gcat = np.concatenate([g1, g2, g3], axis=0)  # [12288 rows, 512 h]
        # pack to [rb, p, hc, col]: gpack[rb, p, hc, col] = gcat[rb*512+col, hc*128+p]
        # scaled x4096 into fp8 e4m3 (rms ~3.6, |max| ~20 — mid-range)
        gpack = np.ascontiguousarray(
            (gcat * GS).reshape(N_RB, 512, KC_H, P).transpose(0, 3, 2, 1)
        ).astype(f8np)
        b1c = np.ascontiguousarray(
            gen_b2[rsl].reshape(R_LOC, DIM).T.reshape(KC_D, P, R_LOC))
        b2c = np.ascontiguousarray(
            gen_b2[RD + c * R_LOC * DIM: RD + (c + 1) * R_LOC * DIM]
            .reshape(R_LOC, DIM).T.reshape(KC_D, P, R_LOC))
        # hmg carries 1/4096 (folded into gate); compensate the bias-corr
        # matmul operand
        b3ct = np.ascontiguousarray(b3_rmaj[:, c * R_LOC:(c + 1) * R_LOC].T) * GS
        hsl = slice(c * HID_LOC, (c + 1) * HID_LOC)
        in_maps.append({
            "x": xs,
            "xt": xt,
            "mtt": mtt,
            "gw1t": gw1t,
            "gb1": np.asarray(gen_b1, f4),
            "g2p": gpack,
            "b1c": b1c,
            "b2c": b2c,
            "b3ct": b3ct,
            "w1t": np.ascontiguousarray(W1[hsl].T),
            "w2t": np.ascontiguousarray(W2[hsl].T),
            "w3t": np.ascontiguousarray(W3[:, hsl].T),
            "bb1": np.asarray(b1, f4)[hsl],
            "bb2": np.asarray(b2, f4)[hsl],
            "b3": np.asarray(b3, f4),
            "gate": gate,
        })
    return in_maps


def kernel(**inputs):
    from concourse.bass_utils import run_bass_kernel_spmd

    gen_bias_zero = not np.any(np.asarray(inputs["gen_b2"]))
    b3_zero = not np.any(np.asarray(inputs["b3"]))
    key = ("nc", gen_bias_zero, b3_zero)
    if key not in _CACHE:
        _CACHE[key] = _build_program(gen_bias_zero=gen_bias_zero,
                                     b3_zero=b3_zero)
    nc = _CACHE[key]

    in_maps = _prep_inputs(**{k: np.asarray(v) for k, v in inputs.items()})
    res = run_bass_kernel_spmd(nc, in_maps, core_ids=list(range(NCORES)))
    import os as _os
    if bool(int(_os.environ.get("KERNEL_SKIP_RS", "0"))):
        y = sum(res.results[c]["y_out"] for c in range(NCORES))
    else:
        Y = np.stack([res.results[c]["y_out"] for c in range(NCORES)])
        if int(_os.environ.get("KERNEL_RS_CHUNKS", "8")) == 1:
            y = Y.reshape(NTOK, DIM)
        else:
            # chunked RS: core c's rows [16 tb : 16 tb + 16] hold tokens
            # 128 tb + 16 c + [0, 16)
            y = Y.reshape(NCORES, NTB, P // NCORES, DIM).transpose(
                1, 0, 2, 3).reshape(NTOK, DIM)
    return y.reshape(B, T, DIM).astype(np.float32)



# revision 35
# speedup vs baseline: 1.5907x; 1.0629x over previous
"""Trainium2 Bass kernel for FFNDynamicMemories (SwiGLU FFN + per-token
hypernetwork-generated low-rank memory path).

Sharding strategy (8 cores):
  - The dominant cost is gen_w2 [3*R*DIM=98304, GEN_HIDDEN=512] (192 MiB fp32):
    it is sharded across cores by *generated-weight rows*: core c owns
    r in [8c, 8c+8) of W1_m and W2_m rows, plus the matching r-slice of W3_m
    (reordered r-major on the host so each 512-row block is one r).
    Each core therefore reads only 1/8 of gen_w2.
  - hg = silu(gen_w1 @ m_tok) is computed by every core for all tokens (tiny).
  - The base SwiGLU FFN is sharded by hidden dim (256 of 2048 per core).
  - Each core produces a full-shape partial y; one on-device ReduceScatter
    (over the token axis) combines them; the host just concatenates the
    8 [128, 512] shards.

Compute mapping per core:
  - Generation matmuls run in fp8e4 (e4m3) with MatmulPerfMode.DoubleRow
    (2 K-rows per free-dim slot, ~1.4x the bf16/fp32r PE rate at FD=512,
    and 4x less HBM traffic for the gen_w2 stream).  gen_w2 is host-scaled
    by 4096 so its ~1e-3-rms values land mid-range in e4m3; the u-path
    compensates with x/4096 (host) and the y_mem path with
    sigmoid(gate)/4096 folded into the per-token h_mem scalars.  The
    mem-path output is ~0.1% of the final scale, so fp8's ~4% error there
    contributes ~3e-4 relative — same order as the fp32r baseline.
    w_tile[t_block 128, 512 rows] = hgT_pair.T @ gen_w2T_pair, accumulated
    over 2 h-pair-chunks (each K=256) in PSUM.
  - The per-token "apply" (u1/u2 row-dot with x, y_mem accumulation over r)
    uses fused DVE ops reading the generated tile straight from PSUM:
      u[t, r]   = tensor_tensor_reduce(w_psum * x, +, init=x@gen_b2_slice)
      y3acc[t]  += scalar_tensor_tensor(w_psum * h_mem[t, r]) (in-place add)
  - gen_w2 row-blocks stream in the OUTER loop (tokens inner) so the big
    tensor is read from HBM exactly once.
"""

import sys

sys.path.insert(0, "/opt/trn_rl_repo")

import numpy as np

# Problem constants (hardcoded per the harness contract).
B, T, DIM, HIDDEN, D_M, R = 2, 512, 512, 2048, 256, 64
GEN_HIDDEN = 2 * D_M  # 512
NTOK = B * T  # 1024
RD = R * DIM  # 32768
NCORES = 8
R_LOC = R // NCORES  # 8 r values per core
HID_LOC = HIDDEN // NCORES  # 256 hidden per core
P = 128
NTB = NTOK // P  # 8 token blocks
N_RB = 3 * R_LOC  # 24 row-blocks of 512 generated rows per core
KC_H = GEN_HIDDEN // P  # 4 h-chunks
KC_D = DIM // P  # 4 d-chunks
KC_DM = D_M // P  # 2 d_m chunks
KC_HID = HID_LOC // P  # 2 hidden chunks

_CACHE = {}


def _build_program(gen_bias_zero=False, b3_zero=False):
    """gen_bias_zero/b3_zero: value-specialization flags (checked by kernel()
    against the actual inputs; a mismatch rebuilds the program). When gen_b2
    is all-zero the u/y_mem bias-correction matmuls, the h_mem transpose and
    the y3corr matmul are elided; when b3 is all-zero its K=1 row matmul is."""
    import concourse.bass as bass
    import concourse.mybir as mybir
    import concourse.tile as tile
    from concourse import bacc
    from concourse.masks import make_identity

    f32 = mybir.dt.float32
    f32r = mybir.dt.float32r
    f8 = mybir.dt.float8e4
    DR = mybir.MatmulPerfMode.DoubleRow
    Act = mybir.ActivationFunctionType
    Alu = mybir.AluOpType

    nc = bacc.Bacc("TRN2", target_bir_lowering=False, debug=False,
                   num_devices=NCORES)

    # ---- DRAM parameters (per-core shapes) ----
    bf16 = mybir.dt.bfloat16
    x_d = nc.dram_tensor("x", [NTOK, DIM], bf16, kind="ExternalInput")
    xt_d = nc.dram_tensor("xt", [DIM, NTOK], f32r, kind="ExternalInput")
    mtt_d = nc.dram_tensor("mtt", [D_M, NTOK], bf16, kind="ExternalInput")
    gw1t_d = nc.dram_tensor("gw1t", [D_M, GEN_HIDDEN], bf16, kind="ExternalInput")
    gb1_d = nc.dram_tensor("gb1", [GEN_HIDDEN], f32, kind="ExternalInput")
    # packed gen_w2 shard: [rb, p(h within chunk), hc, 512 rows], fp8 x4096
    g2p_d = nc.dram_tensor("g2p", [N_RB, P, KC_H, 512], f8, kind="ExternalInput")
    b1c_d = nc.dram_tensor("b1c", [KC_D, P, R_LOC], f32r, kind="ExternalInput")
    b2c_d = nc.dram_tensor("b2c", [KC_D, P, R_LOC], f32r, kind="ExternalInput")
    b3ct_d = nc.dram_tensor("b3ct", [R_LOC, DIM], f32r, kind="ExternalInput")
    w1t_d = nc.dram_tensor("w1t", [DIM, HID_LOC], f32r, kind="ExternalInput")
    w2t_d = nc.dram_tensor("w2t", [DIM, HID_LOC], f32r, kind="ExternalInput")
    w3t_d = nc.dram_tensor("w3t", [HID_LOC, DIM], f32r, kind="ExternalInput")
    bb1_d = nc.dram_tensor("bb1", [HID_LOC], f32, kind="ExternalInput")
    bb2_d = nc.dram_tensor("bb2", [HID_LOC], f32, kind="ExternalInput")
    b3_d = nc.dram_tensor("b3", [DIM], f32, kind="ExternalInput")
    gate_d = nc.dram_tensor("gate", [1, 1], f32, kind="ExternalInput")
    import os as _os
    _skip_rs = bool(int(_os.environ.get("KERNEL_SKIP_RS", "0")))
    _exact_silu = bool(int(_os.environ.get("KERNEL_EXACT_SILU", "0")))
    # ReduceScatter chunking: 8 = one collective per finished token block
    # (overlaps all but the last chunk with pass-B compute); 1 = single RS
    _rs_chunks = int(_os.environ.get("KERNEL_RS_CHUNKS", "8"))
    RS_O = P // NCORES  # 16 output rows per core per RS chunk
    # A/B probe: disable DoubleRow (plain fp8 matmuls, 4 K-chunks) to see
    # whether the real PE rate binds the measured time
    _no_dr = bool(int(_os.environ.get("KERNEL_NO_DR", "0")))
    # Bottleneck probes (break correctness, timing only): emit half the gen
    # matmul work / half-width apply ops
    _half_gen = bool(int(_os.environ.get("KERNEL_HALF_GEN", "0")))
    _half_stt = bool(int(_os.environ.get("KERNEL_HALF_STT", "0")))
    if _skip_rs:
        y_out_d = nc.dram_tensor("y_out", [NTOK, DIM], f32, kind="ExternalOutput")
    else:
        y_out_d = nc.dram_tensor("y_out", [P, DIM], f32, kind="ExternalOutput")

    with tile.TileContext(nc) as tc:
        with (
            tc.tile_pool(name="const", bufs=1) as cpool,
            tc.tile_pool(name="g2s", bufs=4) as g2pool,
            tc.tile_pool(name="scr", bufs=3) as spool,
            tc.tile_pool(name="outp", bufs=3) as opool,
            tc.tile_pool(name="dram", bufs=1, space="DRAM") as dpool,
        ):
            # ---- resident SBUF tiles ----
            x_sb = cpool.tile([P, NTB, DIM], bf16)
            xt_sb = cpool.tile([P, KC_D, NTOK], f32r)
            mtt_sb = cpool.tile([P, KC_DM, NTOK], bf16)
            gw1t_sb = cpool.tile([P, KC_DM, GEN_HIDDEN], bf16)
            gb1_sb = cpool.tile([P, KC_H], f32)
            w1t_sb = cpool.tile([P, KC_D, HID_LOC], f32r)
            w2t_sb = cpool.tile([P, KC_D, HID_LOC], f32r)
            w3t_sb = cpool.tile([P, KC_HID, DIM], f32r)
            bb1_sb = cpool.tile([P, KC_HID], f32)
            bb2_sb = cpool.tile([P, KC_HID], f32)
            b3_sb = cpool.tile([1, DIM], f32)
            b3r_sb = cpool.tile([1, DIM], f32r)
            b1c_sb = cpool.tile([P, KC_D, R_LOC], f32r)
            b2c_sb = cpool.tile([P, KC_D, R_LOC], f32r)
            b3ct_sb = cpool.tile([R_LOC, DIM], f32r)
            gate_sb = cpool.tile([P, 1], f32)
            ident_sb = cpool.tile([P, P], f32)
            ones1f_sb = cpool.tile([1, P], f32)
            ones1_sb = cpool.tile([1, P], f32r)
            hgt_sb = cpool.tile([P, KC_H, NTOK], f8)
            ht_sb = cpool.tile([P, KC_HID, NTOK], f32r)
            ub1_sb = cpool.tile([P, NTB, R_LOC], f32)
            ub2_sb = cpool.tile([P, NTB, R_LOC], f32)
            u1_sb = cpool.tile([P, NTB, R_LOC], f32)
            u2_sb = cpool.tile([P, NTB, R_LOC], f32)
            hmg_sb = cpool.tile([P, NTB, R_LOC], f32)
            hmt_sb = cpool.tile([R_LOC, NTB, P], f32r)
            # y3 accumulates in bf16: the ACT engine drains each pass-B psum
            # tile (scaled by hmg) to bf16, and DVE adds run at the 2x
            # 16-bit rate; y_mem is ~0.1% of the output so bf16 is harmless
            y3_sb = cpool.tile([P, NTB, DIM], mybir.dt.bfloat16)
            # all 8 pass-B gen tiles stay resident (2 MiB fp8) so pass B can
            # run tb-outer and finish whole token blocks early
            g3_sb = cpool.tile([P, R_LOC, KC_H, 512], f8)

            # All resident loads go on the ACT HWDGE ring, ordered by first
            # use (hgT inputs first); the SP ring is reserved for the g2p
            # stream so the first gen tile lands within ~3us.
            mtt_r = mtt_d.ap().rearrange("(kc p) t -> p kc t", p=P)
            for kc in range(KC_DM):
                nc.sync.dma_start(out=mtt_sb[:, kc, :], in_=mtt_r[:, kc, :])
            nc.scalar.dma_start(out=gw1t_sb[:], in_=gw1t_d.ap().rearrange("(kc p) h -> p kc h", p=P))
            nc.scalar.dma_start(out=gb1_sb[:], in_=gb1_d.ap().rearrange("(hc p) -> p hc", p=P))
            # First gen-weight tile DMAs are hoisted ahead of the resident
            # bulk (SP ring) so the gen stream can start as soon as hgT is
            # ready; x is interleaved right behind since pass A's reduce
            # consumes it within a few us of the first gen matmul.
            # (Loop-bench mode can't hoist: the slots would never recycle.)
            bench_iters = int(_os.environ.get("KERNEL_BENCH_ITERS", "1"))
            N_HOIST = 4 if bench_iters == 1 else 0
            gt_hoist = []
            x_r = x_d.ap().rearrange("(tb p) d -> p tb d", p=P)
            for rb in range(N_HOIST):
                gth = g2pool.tile([P, KC_H, 512], f8, tag="gt")
                nc.sync.dma_start(out=gth[:], in_=g2p_d[rb])
                gt_hoist.append(gth)
                if rb == 0:
                    for tb in range(3):
                        nc.sync.dma_start(out=x_sb[:, tb, :], in_=x_r[:, tb, :])
                elif rb == 1:
                    for tb in range(3, NTB):
                        nc.sync.dma_start(out=x_sb[:, tb, :], in_=x_r[:, tb, :])
            if N_HOIST == 0:
                for tb in range(NTB):
                    nc.sync.dma_start(out=x_sb[:, tb, :], in_=x_r[:, tb, :])
            # Everything needed only after pass A rides the idle GPSIMD
            # SWDGE ring.
            nc.gpsimd.dma_start(out=gate_sb[:], in_=gate_d.ap().to_broadcast((P, 1)))
            xt_r = xt_d.ap().rearrange("(kc p) t -> p kc t", p=P)
            for kc in range(KC_D):
                nc.gpsimd.dma_start(out=xt_sb[:, kc, :], in_=xt_r[:, kc, :])
            w1t_r = w1t_d.ap().rearrange("(kc p) h -> p kc h", p=P)
            w2t_r = w2t_d.ap().rearrange("(kc p) h -> p kc h", p=P)
            w3t_r = w3t_d.ap().rearrange("(kc p) d -> p kc d", p=P)
            for kc in range(KC_D):
                nc.gpsimd.dma_start(out=w1t_sb[:, kc, :], in_=w1t_r[:, kc, :])
                nc.gpsimd.dma_start(out=w2t_sb[:, kc, :], in_=w2t_r[:, kc, :])
            nc.gpsimd.dma_start(out=bb1_sb[:], in_=bb1_d.ap().rearrange("(hc p) -> p hc", p=P))
            nc.gpsimd.dma_start(out=bb2_sb[:], in_=bb2_d.ap().rearrange("(hc p) -> p hc", p=P))
            if not gen_bias_zero:
                nc.gpsimd.dma_start(out=b1c_sb[:], in_=b1c_d.ap().rearrange("kc p r -> p kc r"))
                nc.gpsimd.dma_start(out=b2c_sb[:], in_=b2c_d.ap().rearrange("kc p r -> p kc r"))
                nc.gpsimd.dma_start(out=b3ct_sb[:], in_=b3ct_d.ap())
            if not b3_zero:
                nc.gpsimd.dma_start(out=b3_sb[:1, :], in_=b3_d.ap().unsqueeze(0))
            for kc in range(KC_HID):
                nc.gpsimd.dma_start(out=w3t_sb[:, kc, :], in_=w3t_r[:, kc, :])

            if not gen_bias_zero:
                make_identity(nc, ident_sb[:])
            if not b3_zero:
                nc.vector.memset(ones1f_sb[:1, :], 1.0)
                nc.scalar.activation(ones1_sb[:1, :], ones1f_sb[:1, :], Act.Copy,
                                     bias=0.0, scale=1.0)
                # b3 / 8 (each core contributes an eighth; ReduceScatter sums)
                nc.scalar.activation(b3r_sb[:1, :], b3_sb[:1, :], Act.Copy,
                                     bias=0.0, scale=0.125)

            # ---- hgT = silu(gen_w1 @ m_tok + gen_b1), layout [h, tokens] ----
            # all 4 h-chunk matmul groups run back-to-back on the PE (4 psum
            # bufs), then silus are emitted token-half-major so the halves
            # gating pass A's first DR matmuls complete first
            with tc.tile_pool(name="hgps", bufs=4, space="PSUM") as hgps:
                hg_ps = []
                for hb in range(KC_H):
                    ps = hgps.tile([P, NTOK], f32)
                    hg_ps.append(ps)
                    for tn in range(2):
                        for kc in range(KC_DM):
                            nc.tensor.matmul(
                                ps[:, tn * 512:(tn + 1) * 512],
                                gw1t_sb[:, kc, hb * P:(hb + 1) * P],
                                mtt_sb[:, kc, tn * 512:(tn + 1) * 512],
                                start=(kc == 0), stop=(kc == KC_DM - 1),
                            )
                if _exact_silu:
                    for hb in range(KC_H):
                        ps = hg_ps[hb]
                        # silu(z) = z * sigmoid(z), z = ps + gen_b1 (matches
                        # jax exactly; CoreSim has no Silu LUT); DVE writes a
                        # f32 scratch, ACT converts to the fp8 gen operand
                        sg = spool.tile([P, NTOK], f32, tag="hg_sg")
                        nc.scalar.activation(sg[:], ps[:], Act.Sigmoid,
                                             bias=gb1_sb[:, hb:hb + 1], scale=1.0)
                        sf = spool.tile([P, NTOK], f32, tag="hg_sf")
                        nc.vector.scalar_tensor_tensor(
                            out=sf[:], in0=ps[:],
                            scalar=gb1_sb[:, hb:hb + 1], in1=sg[:],
                            op0=Alu.add, op1=Alu.mult,
                        )
                        nc.scalar.activation(hgt_sb[:, hb, :], sf[:], Act.Copy,
                                             bias=0.0, scale=1.0)
                else:
                    for tn in range(2):
                        tsl = slice(tn * 512, (tn + 1) * 512)
                        for hb in range(KC_H):
                            nc.scalar.activation(hgt_sb[:, hb, tsl],
                                                 hg_ps[hb][:, tsl], Act.Silu,
                                                 bias=gb1_sb[:, hb:hb + 1],
                                                 scale=1.0)

            cc_in = dpool.tile([NTOK, DIM], f32)
            cc_out = dpool.tile([P, DIM], f32)

            with (
                tc.tile_pool(name="genps", bufs=5, space="PSUM") as genps,
                tc.tile_pool(name="trps", bufs=1, space="PSUM") as trps,
                tc.tile_pool(name="ybps", bufs=2, space="PSUM") as ybps,
            ):
                def emit_hmem(tb):
                    # h_mem[t, r] = gate * silu(u1) * u2 for one token block
                    s = spool.tile([P, R_LOC], f32, tag="hm_s")
                    if _exact_silu:
                        nc.scalar.activation(s[:], u1_sb[:, tb, :], Act.Sigmoid)
                        nc.vector.tensor_mul(s[:], s[:], u1_sb[:, tb, :])
                    else:
                        nc.scalar.activation(s[:], u1_sb[:, tb, :], Act.Silu)
                    nc.vector.tensor_mul(s[:], s[:], u2_sb[:, tb, :])
                    # gate_sb = sigmoid(mem_gate)/4096 (host-computed); the
                    # /4096 cancels the gen_w2 fp8 pre-scale on the pass-B
                    # psum tiles
                    nc.vector.tensor_scalar_mul(hmg_sb[:, tb, :], s[:],
                                                gate_sb[:, :1])

                def main_body():
                    # -- pass A: generate W1_m / W2_m row-blocks, reduce vs x --
                    for rb in range(2 * R_LOC):  # 0..7 -> W1_m r, 8..15 -> W2_m r
                        mat, r = divmod(rb, R_LOC)
                        if rb < N_HOIST:
                            gt = gt_hoist[rb]
                        else:
                            gt = g2pool.tile([P, KC_H, 512], f8, tag="gt")
                            nc.sync.dma_start(out=gt[:], in_=g2p_d[rb])
                        u_sb = u1_sb if mat == 0 else u2_sb
                        for tb in range(NTB):
                            wps = genps.tile([P, 512], f32, tag="wps")
                            if _no_dr:
                                for hc in range(KC_H):
                                    nc.tensor.matmul(
                                        wps[:],
                                        hgt_sb[:, hc, tb * P:(tb + 1) * P],
                                        gt[:, hc, :],
                                        start=(hc == 0), stop=(hc == KC_H - 1),
                                    )
                            else:
                                npc = 1 if _half_gen else KC_H // 2
                                for pc in range(npc):
                                    nc.tensor.matmul(
                                        wps[:],
                                        hgt_sb[:, 2 * pc:2 * pc + 2, tb * P:(tb + 1) * P],
                                        gt[:, 2 * pc:2 * pc + 2, :],
                                        start=(pc == 0), stop=(pc == npc - 1),
                                        perf_mode=DR,
                                    )
                            W = 256 if _half_stt else 512
                            scr = spool.tile([P, 512], f32, tag="scr")
                            # u[t, r] = sum_d w[t, (r, d)] * x[t, d]  (bias added
                            # later; tensor_tensor_reduce would fuse it but
                            # crashes on HW)
                            nc.vector.scalar_tensor_tensor(
                                out=scr[:, :W], in0=wps[:, :W], scalar=1.0,
                                in1=x_sb[:, tb, :W],
                                op0=Alu.mult, op1=Alu.mult,
                                accum_out=u_sb[:, tb, r:r + 1],
                            )
                            if rb == 2 * R_LOC - 1 and gen_bias_zero:
                                # u1/u2 for this tb are complete: emit its
                                # h_mem chain right away so pass B's drains
                                # aren't gated by a serial h_mem block
                                emit_hmem(tb)

                    # pass-B gen tiles stream behind the pass-A tiles on the
                    # same ring; they are resident by the time pass B starts
                    for r3 in range(R_LOC):
                        nc.sync.dma_start(out=g3_sb[:, r3],
                                          in_=g2p_d[2 * R_LOC + r3])

                    # -- base path: hT = silu(W1 x + b1) * (W2 x + b2) --
                    # (emitted after pass A so the PE doesn't stall on xt/w1t
                    # loads before reaching the gen stream; psum tiles share
                    # the wps rotation)
                    for hc in range(KC_HID):
                        for tn in range(2):
                            tsl = slice(tn * 512, (tn + 1) * 512)
                            ps1 = genps.tile([P, 512], f32, tag="wps")
                            ps2 = genps.tile([P, 512], f32, tag="wps")
                            for kc in range(KC_D):
                                nc.tensor.matmul(
                                    ps1[:], w1t_sb[:, kc, hc * P:(hc + 1) * P],
                                    xt_sb[:, kc, tsl],
                                    start=(kc == 0), stop=(kc == KC_D - 1),
                                )
                                nc.tensor.matmul(
                                    ps2[:], w2t_sb[:, kc, hc * P:(hc + 1) * P],
                                    xt_sb[:, kc, tsl],
                                    start=(kc == 0), stop=(kc == KC_D - 1),
                                )
                            s1 = spool.tile([P, 512], f32, tag="scr")
                            if _exact_silu:
                                sg1 = spool.tile([P, 512], f32, tag="scr")
                                nc.scalar.activation(sg1[:], ps1[:], Act.Sigmoid,
                                                     bias=bb1_sb[:, hc:hc + 1],
                                                     scale=1.0)
                                nc.vector.scalar_tensor_tensor(
                                    out=s1[:], in0=ps1[:],
                                    scalar=bb1_sb[:, hc:hc + 1], in1=sg1[:],
                                    op0=Alu.add, op1=Alu.mult,
                                )
                            else:
                                nc.scalar.activation(s1[:], ps1[:], Act.Silu,
                                                     bias=bb1_sb[:, hc:hc + 1],
                                                     scale=1.0)
                            nc.vector.scalar_tensor_tensor(
                                out=ht_sb[:, hc, tsl], in0=ps2[:],
                                scalar=bb2_sb[:, hc:hc + 1], in1=s1[:],
                                op0=Alu.add, op1=Alu.mult,
                            )

                    # -- u-path bias terms: ub{1,2}[t, r] = x[t] @ gen_b2_blk --
                    if not gen_bias_zero:
                        for tb in range(NTB):
                            uc1 = genps.tile([P, R_LOC], f32, tag="wps")
                            uc2 = genps.tile([P, R_LOC], f32, tag="wps")
                            for kc in range(KC_D):
                                nc.tensor.matmul(
                                    uc1[:], xt_sb[:, kc, tb * P:(tb + 1) * P],
                                    b1c_sb[:, kc, :],
                                    start=(kc == 0), stop=(kc == KC_D - 1))
                                nc.tensor.matmul(
                                    uc2[:], xt_sb[:, kc, tb * P:(tb + 1) * P],
                                    b2c_sb[:, kc, :],
                                    start=(kc == 0), stop=(kc == KC_D - 1))
                            nc.scalar.activation(ub1_sb[:, tb, :], uc1[:], Act.Copy,
                                                 bias=0.0, scale=1.0)
                            nc.scalar.activation(ub2_sb[:, tb, :], uc2[:], Act.Copy,
                                                 bias=0.0, scale=1.0)

                    # -- h_mem = g * silu(u1 + ub1) * (u2 + ub2), + transpose --
                    # (with zero gen bias this was already emitted inside
                    # pass A's last row-block, per token block)
                    if not gen_bias_zero:
                        for tb in range(NTB):
                            nc.vector.tensor_add(u1_sb[:, tb, :], u1_sb[:, tb, :],
                                                 ub1_sb[:, tb, :])
                            nc.vector.tensor_add(u2_sb[:, tb, :], u2_sb[:, tb, :],
                                                 ub2_sb[:, tb, :])
                            emit_hmem(tb)
                            tp = trps.tile([R_LOC, P], f32, tag="tp")
                            nc.tensor.transpose(tp[:], hmg_sb[:, tb, :], ident_sb[:])
                            nc.scalar.activation(hmt_sb[:, tb, :], tp[:], Act.Copy,
                                                 bias=0.0, scale=1.0)

                    def emit_final(tb):
                        # yb = W3 h (+ b3/8 + y_mem bias corr); out = yb + y3acc
                        yb = ybps.tile([P, DIM], f32, tag="yb")
                        n_mm = KC_HID + (not b3_zero) + (not gen_bias_zero)
                        mm_i = 0
                        for hc in range(KC_HID):
                            mm_i += 1
                            nc.tensor.matmul(yb[:], ht_sb[:, hc, tb * P:(tb + 1) * P],
                                             w3t_sb[:, hc, :],
                                             start=(hc == 0), stop=(mm_i == n_mm))
                        if not b3_zero:
                            mm_i += 1
                            nc.tensor.matmul(yb[:], ones1_sb[:1, :], b3r_sb[:1, :],
                                             start=False, stop=(mm_i == n_mm))
                        if not gen_bias_zero:
                            mm_i += 1
                            nc.tensor.matmul(yb[:], hmt_sb[:, tb, :], b3ct_sb[:],
                                             start=False, stop=(mm_i == n_mm))
                        out_t = opool.tile([P, DIM], f32, tag="out_t")
                        nc.vector.tensor_add(out_t[:], yb[:], y3_sb[:, tb, :])
                        nc.sync.dma_start(out=cc_in[tb * P:(tb + 1) * P, :],
                                          in_=out_t[:])

                    # -- pass B: tb-outer so each token block (and its RS
                    #    chunk) completes early.  Per (tb, r): ACT drains the
                    #    generated psum tile scaled by hmg[t, r] (per-token
                    #    scale) to bf16; DVE accumulates y3 with 2x-rate
                    #    16-bit adds.  r == 0 initializes y3 via the drain
                    #    itself --
                    for tb in range(NTB):
                        for r in range(R_LOC):
                            wps = genps.tile([P, 512], f32, tag="wps")
                            if _no_dr:
                                for hc in range(KC_H):
                                    nc.tensor.matmul(
                                        wps[:],
                                        hgt_sb[:, hc, tb * P:(tb + 1) * P],
                                        g3_sb[:, r, hc, :],
                                        start=(hc == 0), stop=(hc == KC_H - 1),
                                    )
                            else:
                                npc = 1 if _half_gen else KC_H // 2
                                for pc in range(npc):
                                    nc.tensor.matmul(
                                        wps[:],
                                        hgt_sb[:, 2 * pc:2 * pc + 2, tb * P:(tb + 1) * P],
                                        g3_sb[:, r, 2 * pc:2 * pc + 2, :],
                                        start=(pc == 0), stop=(pc == npc - 1),
                                        perf_mode=DR,
                                    )
                            if r == 0:
                                nc.scalar.activation(
                                    y3_sb[:, tb, :], wps[:], Act.Copy,
                                    bias=0.0, scale=hmg_sb[:, tb, r:r + 1])
                            else:
                                s16 = spool.tile([P, 512], mybir.dt.bfloat16,
                                                 tag="s16")
                                nc.scalar.activation(
                                    s16[:], wps[:], Act.Copy,
                                    bias=0.0, scale=hmg_sb[:, tb, r:r + 1])
                                nc.vector.tensor_add(
                                    y3_sb[:, tb, :], y3_sb[:, tb, :], s16[:])
                        emit_final(tb)
                        if not _skip_rs and _rs_chunks > 1:
                            nc.gpsimd.collective_compute(
                                "ReduceScatter",
                                mybir.AluOpType.add,
                                replica_groups=[list(range(NCORES))],
                                ins=[cc_in[tb * P:(tb + 1) * P, :].opt()],
                                outs=[cc_out[tb * RS_O:(tb + 1) * RS_O, :].opt()],
                            )

                if bench_iters > 1:
                    with tc.For_i(0, bench_iters, 1):
                        main_body()
                else:
                    main_body()

                if _skip_rs:
                    nc.sync.dma_start(out=y_out_d.ap(), in_=cc_in[:])
                else:
                    if _rs_chunks == 1:
                        nc.gpsimd.collective_compute(
                            "ReduceScatter",
                            mybir.AluOpType.add,
                            replica_groups=[list(range(NCORES))],
                            ins=[cc_in.opt()],
                            outs=[cc_out.opt()],
                        )
                    nc.sync.dma_start(out=y_out_d.ap(), in_=cc_out[:])

    nc.compile()
    return nc


def _prep_inputs(x, m_tok, W1, W2, W3, b1, b2, b3, gen_w1, gen_b1, gen_w2,
                 gen_b2, mem_gate):
    """Shard + relayout full inputs into 8 per-core input maps (numpy only)."""
    import ml_dtypes
    f4 = np.float32
    f8np = ml_dtypes.float8_e4m3
    GS = np.float32(4096.0)  # gen_w2 fp8 pre-scale (power of 2: exact inverse)
    bf = ml_dtypes.bfloat16
    x2d = np.ascontiguousarray(x.reshape(NTOK, DIM), dtype=f4)
    xt = np.ascontiguousarray(x2d.T)
    # pass-A reduce operand (bf16; cancels the gen pre-scale)
    xs = (x2d * (1.0 / GS)).astype(bf)
    mtt = np.ascontiguousarray(m_tok.reshape(NTOK, D_M).T.astype(bf))
    gw1t = np.ascontiguousarray(np.asarray(gen_w1, f4).T.astype(bf))
    gen_w2 = np.asarray(gen_w2, f4)
    gen_b2 = np.asarray(gen_b2, f4)
    # gate carries sigmoid (host-computed scalar) and the pass-B descale
    gate = (1.0 / (1.0 + np.exp(-np.asarray(mem_gate, np.float64))) / f4(GS))
    gate = np.asarray(gate, f4).reshape(1, 1)
    W1 = np.asarray(W1, f4)
    W2 = np.asarray(W2, f4)
    W3 = np.asarray(W3, f4)

    # W3_m block of gen_w2 reordered r-major: [R, DIM, GEN_HIDDEN]
    g3_rmaj = gen_w2[2 * RD:].reshape(DIM, R, GEN_HIDDEN).transpose(1, 0, 2)
    b3_rmaj = gen_b2[2 * RD:].reshape(DIM, R)  # [d, r]

    in_maps = []
    for c in range(NCORES):
        rsl = slice(c * R_LOC * DIM, (c + 1) * R_LOC * DIM)
        g1 = gen_w2[rsl]
        g2 = gen_w2[RD + c * R_LOC * DIM: RD + (c + 1) * R_LOC * DIM]
        g3 = g3_rmaj[c * R_LOC:(c + 1) * R_LOC].reshape(R_LOC * DIM, GEN_HIDDEN)
        gcat = np.concatenate([g1, g2, g3], axis=0)  # [12288 rows, 512 h]
        # pack to [rb, p, hc, col]: gpack[rb, p, hc, col] = gcat[rb*512+col, hc*128+p]
        # scaled x4096 into fp8 e4m3 (rms ~3.6, |max| ~20 — mid-range)
        gpack = np.ascontiguousarray(
            (gcat * GS).reshape(N_RB, 512, KC_H, P).transpose(0, 3, 2, 1)
        ).astype(f8np)
        b1c = np.ascontiguousarray(
            gen_b2[rsl].reshape(R_LOC, DIM).T.reshape(KC_D, P, R_LOC))
        b2c = np.ascontiguousarray(
            gen_b2[RD + c * R_LOC * DIM: RD + (c + 1) * R_LOC * DIM]
            .reshape(R_LOC, DIM).T.reshape(KC_D, P, R_LOC))
        # hmg carries 1/4096 (folded into gate); compensate the bias-corr
        # matmul operand
        b3ct = np.ascontiguousarray(b3_rmaj[:, c * R_LOC:(c + 1) * R_LOC].T) * GS
        hsl = slice(c * HID_LOC, (c + 1) * HID_LOC)
        in_maps.append({
            "x": xs,
            "xt": xt,
            "mtt": mtt,
            "gw1t": gw1t,
            "gb1": np.asarray(gen_b1, f4),
            "g2p": gpack,
            "b1c": b1c,
            "b2c": b2c,
            "b3ct": b3ct,
            "w1t": np.ascontiguousarray(W1[hsl].T),
            "w2t": np.ascontiguousarray(W2[hsl].T),
            "w3t": np.ascontiguousarray(W3[:, hsl].T),
            "bb1": np.asarray(b1, f4)[hsl],
            "bb2": np.asarray(b2, f4)[hsl],
            "b3": np.asarray(b3, f4),
            "gate": gate,
        })
    return in_maps


def kernel(**inputs):
    from concourse.bass_utils import run_bass_kernel_spmd

    gen_bias_zero = not np.any(np.asarray(inputs["gen_b2"]))
    b3_zero = not np.any(np.asarray(inputs["b3"]))
    key = ("nc", gen_bias_zero, b3_zero)
    if key not in _CACHE:
        _CACHE[key] = _build_program(gen_bias_zero=gen_bias_zero,
                                     b3_zero=b3_zero)
    nc = _CACHE[key]

    in_maps = _prep_inputs(**{k: np.asarray(v) for k, v in inputs.items()})
    res = run_bass_kernel_spmd(nc, in_maps, core_ids=list(range(NCORES)))
    import os as _os
    if bool(int(_os.environ.get("KERNEL_SKIP_RS", "0"))):
        y = sum(res.results[c]["y_out"] for c in range(NCORES))
    else:
        Y = np.stack([res.results[c]["y_out"] for c in range(NCORES)])
        if int(_os.environ.get("KERNEL_RS_CHUNKS", "8")) == 1:
            y = Y.reshape(NTOK, DIM)
        else:
            # chunked RS: core c's rows [16 tb : 16 tb + 16] hold tokens
            # 128 tb + 16 c + [0, 16)
            y = Y.reshape(NCORES, NTB, P // NCORES, DIM).transpose(
                1, 0, 2, 3).reshape(NTOK, DIM)
    return y.reshape(B, T, DIM).astype(np.float32)

